# revision 7
# baseline (speedup 1.0000x reference)
"""Trainium2 Bass kernel for nn_Densenet_with_skip (gauss blur -> sobel ->
angle-binned 8-direction NMS -> gate).

Reformulation (validated vs reference at 1.7e-7 in fp32 numpy):
  b  = gauss5x5(x)                      (zero pad; separable, rank-1)
  gx = (Bv@Sv)^T x (Bh@Dh)              (composed 7-tap bands, replicate pad on b)
  gy = (Bv@Dv)^T x (Bh@Sh)
  bin: m0 = (t1*|gx| >= |gy|)  -> horizontal pair
       m2 = (t1*|gy| >  |gx|)  -> vertical pair
       else diag: gx*gy<0 -> anti-diag pair, else main-diag pair
  out = x * relu(cen*b + v*max(pair))   (pair neighbors, 0 outside image)

All convolutions are done on the TensorEngine as banded matmuls in fp16
(single sweep, ~3.1e-3 rel l2 vs reference). Vertical-pair access is done by
producing row-shifted copies of b (bup/bdn) straight out of the second
(horizontal) matmul pass by shifting the stationary operand's row slice, so
the elementwise stage only ever needs free-dim shifts.
"""

import sys

import numpy as np

sys.path.insert(0, "/opt/trn_rl_repo")

import concourse.bacc as bacc
import concourse.mybir as mybir
from concourse import tile
from concourse.bass_utils import run_bass_kernel_spmd

N = 512
B_TOTAL = 32
N_CORES = 8
B_CORE = B_TOTAL // N_CORES  # 4 images per core
NCHUNK = N // 128  # 4

F16 = mybir.dt.float16
U8 = mybir.dt.uint8
F32 = mybir.dt.float32

T1 = float(np.tan(np.pi / 8))  # tan(22.5 deg)


def _band_ranges(halo):
    """Output-col range [lo, hi) per 128-chunk for a (2*halo+1)-tap band."""
    out = []
    for r in range(NCHUNK):
        lo = max(0, 128 * r - halo)
        hi = min(N, 128 * r + 128 + halo)
        out.append((lo, hi))
    return out


def _banded_mm(nc, psum_ap, lhsT_sl, w_chunks, ranges, out_poff=0):
    """Accumulate sum_r lhsT_r.T @ W_r[:, band] into psum with correct
    PSUM zero-region start/stop handling (each matmul touches either
    all-fresh or all-covered columns)."""
    covered = 0
    n = len(ranges)
    for r in range(n):
        lo, hi = ranges[r]
        first = r == 0
        last = r == n - 1
        if not first and lo < covered:
            nc.tensor.matmul(
                psum_ap[:, lo:covered],
                lhsT_sl[r],
                w_chunks[r][:, lo:covered],
                start=False,
                stop=False,
            )
            lo = covered
        nc.tensor.matmul(
            psum_ap[:, lo:hi],
            lhsT_sl[r],
            w_chunks[r][:, lo:hi],
            start=first,
            stop=last,
        )
        covered = hi


def build_nc(cen, v):
    """Build + compile the per-core program. cen/v: nms kernel center and tap."""
    s = -v  # S = s*b ; z = (cen/s)*S - max(pair of S-neighbors)
    zs = cen / s

    nc = bacc.Bacc("TRN2", target_bir_lowering=False, debug=False)

    x_d = nc.dram_tensor("x", [B_CORE * N, N], F32, kind="ExternalInput").ap()
    w_names = ["bv", "bh", "mvx", "mhx", "mvy", "mhy"]
    w_d = {
        k: nc.dram_tensor("w_" + k, [N, N], F16, kind="ExternalInput").ap()
        for k in w_names
    }
    out_d = nc.dram_tensor("out", [B_CORE * N, N], F32, kind="ExternalOutput").ap()

    r5 = _band_ranges(2)  # 5-tap gauss band
    r7 = _band_ranges(3)  # 7-tap composed sobel band

    with tile.TileContext(nc) as tc:
        with (
            tc.tile_pool(name="wpool", bufs=1) as wpool,
            tc.tile_pool(name="xpool", bufs=1) as xpool,
            tc.tile_pool(name="tT", bufs=2) as tTpool,
            tc.tile_pool(name="post", bufs=2) as post,
            tc.tile_pool(name="outp", bufs=3) as outp,
            tc.tile_pool(name="psumv", bufs=2, space="PSUM") as psumv,
            tc.tile_pool(name="psumh", bufs=1, space="PSUM") as psumh,
        ):
            # --- load weights (band matrices), 4 row-chunks each ---
            w_sb = {}
            for k in w_names:
                w_sb[k] = []
                for r in range(NCHUNK):
                    t = wpool.tile([128, N], F16, tag=f"w_{k}_{r}")
                    nc.sync.dma_start(out=t[:], in_=w_d[k][128 * r : 128 * (r + 1), :])
                    w_sb[k].append(t)

            # --- load x as fp16 (DMA cast) ---
            xh = [[None] * NCHUNK for _ in range(B_CORE)]
            for i in range(B_CORE):
                for r in range(NCHUNK):
                    t = xpool.tile([128, N], F16, tag=f"xh_{i}_{r}")
                    nc.gpsimd.dma_start(
                        out=t[:],
                        in_=x_d[i * N + 128 * r : i * N + 128 * (r + 1), :],
                    )
                    xh[i][r] = t

            for i in range(B_CORE):
                # ---------- vertical passes: tT[k][c] = (W_v^T x)^T ----------
                tT = {}
                for k, wname, rr in (
                    ("b", "bv", r5),
                    ("x", "mvx", r7),
                    ("y", "mvy", r7),
                ):
                    tT[k] = []
                    for c in range(NCHUNK):
                        pv = psumv.tile([128, N], F32, tag="pv")
                        lhsT = [xh[i][r][:, 128 * c : 128 * (c + 1)] for r in range(NCHUNK)]
                        _banded_mm(nc, pv, lhsT, w_sb[wname], rr)
                        st = tTpool.tile([128, N + 2], F16, tag=f"tT_{k}_{c}")
                        nc.gpsimd.memset(st[:, 0:1], 0.0)
                        nc.gpsimd.memset(st[:, N + 1 : N + 2], 0.0)
                        if k == "b":
                            nc.scalar.activation(
                                st[:, 1 : N + 1], pv[:],
                                mybir.ActivationFunctionType.Copy,
                            )
                        else:
                            nc.vector.tensor_copy(out=st[:, 1 : N + 1], in_=pv[:])
                        tT[k].append(st)

                # ---------- horizontal passes + elementwise, per row-tile ----------
                for rt in range(NCHUNK):
                    row0 = 128 * rt

                    def hpass(key, wname, rr, shift, tag):
                        # lhsT rows row0+shift .. row0+shift+128 in padded coords
                        p = psumh.tile([128, N], F32, tag=tag)
                        lo = row0 + 1 + shift
                        lhsT = [tT[key][c][:, lo : lo + 128] for c in range(NCHUNK)]
                        _banded_mm(nc, p, lhsT, w_sb[wname], rr)
                        return p

                    pb = hpass("b", "bh", r5, 0, "pb")
                    pup = hpass("b", "bh", r5, 1, "pup")
                    pdn = hpass("b", "bh", r5, -1, "pdn")
                    pgx = hpass("x", "mhx", r7, 0, "pgx")
                    pgy = hpass("y", "mhy", r7, 0, "pgy")

                    # S/Up/Dn: scaled copies with 1-col zero pad on each side
                    S = post.tile([128, N + 2], F16, tag="S")
                    Up = post.tile([128, N + 2], F16, tag="Up")
                    Dn = post.tile([128, N + 2], F16, tag="Dn")
                    for t in (S, Up, Dn):
                        nc.gpsimd.memset(t[:, 0:1], 0.0)
                        nc.gpsimd.memset(t[:, N + 1 : N + 2], 0.0)
                    nc.scalar.activation(
                        S[:, 1 : N + 1], pb[:], mybir.ActivationFunctionType.Copy,
                        scale=s,
                    )
                    nc.scalar.activation(
                        Up[:, 1 : N + 1], pup[:],
                        mybir.ActivationFunctionType.Copy, scale=s,
                    )
                    nc.vector.tensor_scalar(
                        out=Dn[:, 1 : N + 1], in0=pdn[:],
                        scalar1=s, scalar2=None, op0=mybir.AluOpType.mult,
                    )

                    # masks inputs
                    gys = post.tile([128, N], F16, tag="gys")
                    nc.vector.tensor_copy(out=gys[:], in_=pgy[:])
                    ax = post.tile([128, N], F16, tag="ax")
                    nc.scalar.activation(
                        ax[:], pgx[:], mybir.ActivationFunctionType.Abs
                    )
                    ay = post.tile([128, N], F16, tag="ay")
                    nc.scalar.activation(
                        ay[:], pgy[:], mybir.ActivationFunctionType.Abs
                    )
                    pneg = post.tile([128, N], F32, tag="pneg")
                    nc.vector.tensor_tensor(
                        out=pneg[:], in0=pgx[:], in1=gys[:], op=mybir.AluOpType.mult
                    )
                    mneg = post.tile([128, N], mybir.dt.int32, tag="mneg")
                    nc.vector.tensor_scalar(
                        out=mneg[:], in0=pneg[:].bitcast(mybir.dt.int32),
                        scalar1=31, scalar2=None,
                        op0=mybir.AluOpType.logical_shift_right,
                    )
                    m0 = post.tile([128, N], U8, tag="m0")
                    nc.vector.scalar_tensor_tensor(
                        out=m0[:], in0=ax[:], scalar=T1, in1=ay[:],
                        op0=mybir.AluOpType.mult, op1=mybir.AluOpType.is_ge,
                    )
                    m2 = post.tile([128, N], U8, tag="m2")
                    nc.vector.scalar_tensor_tensor(
                        out=m2[:], in0=ay[:], scalar=T1, in1=ax[:],
                        op0=mybir.AluOpType.mult, op1=mybir.AluOpType.is_gt,
                    )

                    # pair maxes + predicated select (sel starts as main-diag pm1)
                    sel = post.tile([128, N], F16, tag="sel")
                    nc.vector.tensor_tensor(
                        out=sel[:], in0=Dn[:, 0:N], in1=Up[:, 2 : N + 2],
                        op=mybir.AluOpType.max,
                    )
                    pm3 = post.tile([128, N], F16, tag="pm3")
                    nc.vector.tensor_tensor(
                        out=pm3[:], in0=Dn[:, 2 : N + 2], in1=Up[:, 0:N],
                        op=mybir.AluOpType.max,
                    )
                    pm2 = post.tile([128, N], F16, tag="pm2")
                    nc.vector.tensor_tensor(
                        out=pm2[:], in0=Dn[:, 1 : N + 1], in1=Up[:, 1 : N + 1],
                        op=mybir.AluOpType.max,
                    )
                    pm0 = post.tile([128, N], F16, tag="pm0")
                    nc.vector.tensor_tensor(
                        out=pm0[:], in0=S[:, 0:N], in1=S[:, 2 : N + 2],
                        op=mybir.AluOpType.max,
                    )
                    nc.vector.copy_predicated(sel[:], mneg[:], pm3[:])
                    nc.vector.copy_predicated(sel[:], m2[:], pm2[:])
                    nc.vector.copy_predicated(sel[:], m0[:], pm0[:])

                    # z = zs*S - sel ; out = relu(z) * x
                    z = post.tile([128, N], F16, tag="z")
                    nc.vector.scalar_tensor_tensor(
                        out=z[:], in0=S[:, 1 : N + 1], scalar=zs, in1=sel[:],
                        op0=mybir.AluOpType.mult, op1=mybir.AluOpType.subtract,
                    )
                    o = outp.tile([128, N], F32, tag="o")
                    nc.vector.scalar_tensor_tensor(
                        out=o[:], in0=z[:], scalar=0.0, in1=xh[i][rt][:],
                        op0=mybir.AluOpType.max, op1=mybir.AluOpType.mult,
                    )
                    nc.sync.dma_start(
                        out=out_d[i * N + row0 : i * N + row0 + 128, :], in_=o[:]
                    )

    nc.compile()
    return nc


# ---------------------------------------------------------------------------
# host side
# ---------------------------------------------------------------------------

def _make_band(weights, offsets, pad):
    M = np.zeros((N, N), dtype=np.float64)
    for w, o in zip(weights, offsets):
        idx = np.arange(N)
        src = idx + o
        if pad == "replicate":
            np.add.at(M, (np.clip(src, 0, N - 1), idx), w)
        else:
            ok = (src >= 0) & (src < N)
            np.add.at(M, (src[ok], idx[ok]), w)
    return M


def _host_weights(gauss_kernel):
    gk = np.asarray(gauss_kernel, dtype=np.float64)[0, 0]
    U, sv, Vt = np.linalg.svd(gk)
    assert sv[1] < 1e-5 * sv[0], "gauss kernel not rank-1 separable"
    wv = U[:, 0] * np.sqrt(sv[0])
    wh = Vt[0] * np.sqrt(sv[0])
    if wv.sum() < 0:
        wv, wh = -wv, -wh
    o5 = [-2, -1, 0, 1, 2]
    o3 = [-1, 0, 1]
    Bv = _make_band(wv, o5, "zero")
    Bh = _make_band(wh, o5, "zero")
    Sv = _make_band([1, 2, 1], o3, "replicate")
    Dv = _make_band([-1, 0, 1], o3, "replicate")
    Sh = _make_band([1, 2, 1], o3, "replicate")
    Dh = _make_band([-1, 0, 1], o3, "replicate")
    f16 = lambda a: np.ascontiguousarray(a, dtype=np.float16)
    return {
        "w_bv": f16(Bv),
        "w_bh": f16(Bh),
        "w_mvx": f16(Bv @ Sv),
        "w_mhx": f16(Bh @ Dh),
        "w_mvy": f16(Bv @ Dv),
        "w_mhy": f16(Bh @ Sh),
    }


_NC_CACHE = {}
LAST_RESULT = None


def kernel(reconst, gauss_kernel, nms_kernel):
    nk = np.asarray(nms_kernel, dtype=np.float64)
    cen = float(nk[0, 0, 1, 1])
    v = float(nk[0, 0, 1, 2])
    # verify nms kernel structure: center + single tap v per direction
    pos = [(1, 2), (2, 2), (2, 1), (2, 0), (1, 0), (0, 0), (0, 1), (0, 2)]
    for d, (r, c) in enumerate(pos):
        k = nk[d, 0].copy()
        assert abs(k[1, 1] - cen) < 1e-6 and abs(k[r, c] - v) < 1e-6
        k[1, 1] = 0.0
        k[r, c] = 0.0
        assert np.abs(k).max() < 1e-7
    assert v < 0

    key = (round(cen, 9), round(v, 9))
    if key not in _NC_CACHE:
        _NC_CACHE[key] = build_nc(cen, v)
    nc = _NC_CACHE[key]

    w = _host_weights(gauss_kernel)
    x = np.asarray(reconst, dtype=np.float32).reshape(B_TOTAL, N, N)
    in_maps = []
    for core in range(N_CORES):
        m = {"x": np.ascontiguousarray(
            x[core * B_CORE : (core + 1) * B_CORE].reshape(B_CORE * N, N)
        )}
        m.update(w)
        in_maps.append(m)

    res = run_bass_kernel_spmd(nc, in_maps, core_ids=list(range(N_CORES)))
    global LAST_RESULT
    LAST_RESULT = res
    out = np.concatenate(
        [r["out"].reshape(B_CORE, 1, N, N) for r in res.results], axis=0
    )
    return out.astype(np.float32)


# revision 10
# speedup vs baseline: 1.0168x; 1.0168x over previous
"""Trainium2 Bass kernel for nn_Densenet_with_skip (gauss blur -> sobel ->
angle-binned 8-direction NMS -> gate).

Reformulation (validated vs reference at 1.7e-7 in fp32 numpy):
  b  = gauss5x5(x)                      (zero pad; separable, rank-1)
  gx = (Bv@Sv)^T x (Bh@Dh)              (composed 7-tap bands, replicate pad on b)
  gy = (Bv@Dv)^T x (Bh@Sh)
  bin: m0 = (t1*|gx| >= |gy|)  -> horizontal pair
       m2 = (t1*|gy| >  |gx|)  -> vertical pair
       else diag: gx*gy<0 -> anti-diag pair, else main-diag pair
  out = x * relu(cen*b + v*max(pair))   (pair neighbors, 0 outside image)

All convolutions are done on the TensorEngine as banded matmuls in fp16
(single sweep, ~3.1e-3 rel l2 vs reference). Vertical-pair access is done by
producing row-shifted copies of b (bup/bdn) straight out of the second
(horizontal) matmul pass by shifting the stationary operand's row slice, so
the elementwise stage only ever needs free-dim shifts.
"""

import sys

import numpy as np

sys.path.insert(0, "/opt/trn_rl_repo")

import concourse.bacc as bacc
import concourse.mybir as mybir
from concourse import tile
from concourse.bass_utils import run_bass_kernel_spmd

N = 512
B_TOTAL = 32
N_CORES = 8
B_CORE = B_TOTAL // N_CORES  # 4 images per core
NCHUNK = N // 128  # 4

F16 = mybir.dt.float16
U8 = mybir.dt.uint8
F32 = mybir.dt.float32

T1 = float(np.tan(np.pi / 8))  # tan(22.5 deg)


def _band_ranges(halo):
    """Output-col range [lo, hi) per 128-chunk for a (2*halo+1)-tap band."""
    out = []
    for r in range(NCHUNK):
        lo = max(0, 128 * r - halo)
        hi = min(N, 128 * r + 128 + halo)
        out.append((lo, hi))
    return out


def _banded_mm(nc, psum_ap, lhsT_sl, w_chunks, ranges, out_poff=0):
    """Accumulate sum_r lhsT_r.T @ W_r[:, band] into psum with correct
    PSUM zero-region start/stop handling (each matmul touches either
    all-fresh or all-covered columns)."""
    covered = 0
    n = len(ranges)
    for r in range(n):
        lo, hi = ranges[r]
        first = r == 0
        last = r == n - 1
        if not first and lo < covered:
            nc.tensor.matmul(
                psum_ap[:, lo:covered],
                lhsT_sl[r],
                w_chunks[r][:, lo:covered],
                start=False,
                stop=False,
            )
            lo = covered
        nc.tensor.matmul(
            psum_ap[:, lo:hi],
            lhsT_sl[r],
            w_chunks[r][:, lo:hi],
            start=first,
            stop=last,
        )
        covered = hi


def _banded_mm2(nc, psum_ap, lhsT_a, wa, lhsT_b, wb, ranges):
    covered = 0
    n = len(ranges)
    for r in range(n):
        lo, hi = ranges[r]
        first = r == 0
        if not first and lo < covered:
            nc.tensor.matmul(psum_ap[:, lo:covered], lhsT_a[r],
                             wa[r][:, lo:covered], start=False, stop=False)
            lo = covered
        nc.tensor.matmul(psum_ap[:, lo:hi], lhsT_a[r], wa[r][:, lo:hi],
                         start=first, stop=False)
        covered = hi
    for r in range(n):
        lo, hi = ranges[r]
        last = r == n - 1
        nc.tensor.matmul(psum_ap[:, lo:hi], lhsT_b[r], wb[r][:, lo:hi],
                         start=False, stop=last)


def build_nc(cen, v):
    """Build + compile the per-core program. cen/v: nms kernel center and tap."""
    s = -v  # S = s*b ; z = (cen/s)*S - max(pair of S-neighbors)
    zs = cen / s

    nc = bacc.Bacc("TRN2", target_bir_lowering=False, debug=False)

    x_d = nc.dram_tensor("x", [B_CORE * N, N], F32, kind="ExternalInput").ap()
    w_names = ["bv", "bh", "mvx", "mhx", "mvy", "mhy"]
    w_d = {
        k: nc.dram_tensor("w_" + k, [N, N], F16, kind="ExternalInput").ap()
        for k in w_names
    }
    out_d = nc.dram_tensor("out", [B_CORE * N, N], F32, kind="ExternalOutput").ap()

    r5 = _band_ranges(2)  # 5-tap gauss band
    r7 = _band_ranges(3)  # 7-tap composed sobel band

    with tile.TileContext(nc) as tc:
        with (
            tc.tile_pool(name="wpool", bufs=1) as wpool,
            tc.tile_pool(name="xpool", bufs=1) as xpool,
            tc.tile_pool(name="tT", bufs=2) as tTpool,
            tc.tile_pool(name="post", bufs=2) as post,
            tc.tile_pool(name="outp", bufs=3) as outp,
            tc.tile_pool(name="psumv", bufs=2, space="PSUM") as psumv,
            tc.tile_pool(name="psumh", bufs=1, space="PSUM") as psumh,
        ):
            # --- load weights (band matrices), 4 row-chunks each ---
            w_sb = {}
            for k in w_names:
                w_sb[k] = []
                for r in range(NCHUNK):
                    t = wpool.tile([128, N], F16, tag=f"w_{k}_{r}")
                    nc.sync.dma_start(out=t[:], in_=w_d[k][128 * r : 128 * (r + 1), :])
                    w_sb[k].append(t)

            # --- load x as fp16 (DMA cast) ---
            xh = [[None] * NCHUNK for _ in range(B_CORE)]
            for i in range(B_CORE):
                for r in range(NCHUNK):
                    t = xpool.tile([128, N], F16, tag=f"xh_{i}_{r}")
                    nc.gpsimd.dma_start(
                        out=t[:],
                        in_=x_d[i * N + 128 * r : i * N + 128 * (r + 1), :],
                    )
                    xh[i][r] = t

            for i in range(B_CORE):
                # ---------- vertical passes: tT[k][c] = (W_v^T x)^T ----------
                tT = {}
                for k, wname, rr in (
                    ("b", "bv", r5),
                    ("x", "mvx", r7),
                    ("y", "mvy", r7),
                ):
                    tT[k] = []
                    for c in range(NCHUNK):
                        pv = psumv.tile([128, N], F32, tag="pv")
                        lhsT = [xh[i][r][:, 128 * c : 128 * (c + 1)] for r in range(NCHUNK)]
                        _banded_mm(nc, pv, lhsT, w_sb[wname], rr)
                        st = tTpool.tile([128, N + 2], F16, tag=f"tT_{k}_{c}")
                        nc.vector.memset(st[:, 0 : N + 2 : N + 1], 0.0)
                        if k == "b":
                            nc.scalar.activation(
                                st[:, 1 : N + 1], pv[:],
                                mybir.ActivationFunctionType.Copy,
                            )
                        else:
                            nc.vector.tensor_copy(out=st[:, 1 : N + 1], in_=pv[:])
                        tT[k].append(st)

                # ---------- horizontal passes + elementwise, per row-tile ----------
                for rt in range(NCHUNK):
                    row0 = 128 * rt

                    def hpass(key, wname, rr, shift, tag):
                        # lhsT rows row0+shift .. row0+shift+128 in padded coords
                        p = psumh.tile([128, N], F32, tag=tag)
                        lo = row0 + 1 + shift
                        lhsT = [tT[key][c][:, lo : lo + 128] for c in range(NCHUNK)]
                        _banded_mm(nc, p, lhsT, w_sb[wname], rr)
                        return p

                    pb = hpass("b", "bh", r5, 0, "pb")
                    pup = hpass("b", "bh", r5, 1, "pup")
                    pdn = hpass("b", "bh", r5, -1, "pdn")
                    pgx = hpass("x", "mhx", r7, 0, "pgx")
                    pgy = hpass("y", "mhy", r7, 0, "pgy")
                    # gx+gy accumulated in one psum: sign(gx*gy)<0 iff |gx+gy| < max(|gx|,|gy|)
                    pxy = psumh.tile([128, N], F32, tag="pxy")
                    lo = row0 + 1
                    lhsT_x = [tT["x"][c][:, lo : lo + 128] for c in range(NCHUNK)]
                    lhsT_y = [tT["y"][c][:, lo : lo + 128] for c in range(NCHUNK)]
                    _banded_mm2(nc, pxy, lhsT_x, w_sb["mhx"], lhsT_y, w_sb["mhy"], r7)

                    # S/Up/Dn: scaled copies with 1-col zero pad on each side
                    S = post.tile([128, N + 2], F16, tag="S")
                    Up = post.tile([128, N + 2], F16, tag="Up")
                    Dn = post.tile([128, N + 2], F16, tag="Dn")
                    for t in (S, Up, Dn):
                        nc.vector.memset(t[:, 0 : N + 2 : N + 1], 0.0)
                    nc.scalar.activation(
                        S[:, 1 : N + 1], pb[:], mybir.ActivationFunctionType.Copy,
                        scale=s,
                    )
                    nc.scalar.activation(
                        Up[:, 1 : N + 1], pup[:],
                        mybir.ActivationFunctionType.Copy, scale=s,
                    )
                    nc.scalar.activation(
                        Dn[:, 1 : N + 1], pdn[:],
                        mybir.ActivationFunctionType.Copy, scale=s,
                    )

                    # masks inputs
                    ax = post.tile([128, N], F16, tag="ax")
                    nc.scalar.activation(
                        ax[:], pgx[:], mybir.ActivationFunctionType.Abs
                    )
                    ay = post.tile([128, N], F16, tag="ay")
                    nc.scalar.activation(
                        ay[:], pgy[:], mybir.ActivationFunctionType.Abs
                    )
                    pabs = post.tile([128, N], F16, tag="pabs")
                    nc.scalar.activation(
                        pabs[:], pxy[:], mybir.ActivationFunctionType.Abs
                    )
                    mmax = post.tile([128, N], F16, tag="mmax")
                    nc.vector.tensor_tensor(
                        out=mmax[:], in0=ax[:], in1=ay[:], op=mybir.AluOpType.max
                    )
                    mneg = post.tile([128, N], U8, tag="mneg")
                    nc.vector.tensor_tensor(
                        out=mneg[:], in0=pabs[:], in1=mmax[:], op=mybir.AluOpType.is_lt
                    )
                    m0 = post.tile([128, N], U8, tag="m0")
                    nc.vector.scalar_tensor_tensor(
                        out=m0[:], in0=ax[:], scalar=T1, in1=ay[:],
                        op0=mybir.AluOpType.mult, op1=mybir.AluOpType.is_ge,
                    )
                    m2 = post.tile([128, N], U8, tag="m2")
                    nc.vector.scalar_tensor_tensor(
                        out=m2[:], in0=ay[:], scalar=T1, in1=ax[:],
                        op0=mybir.AluOpType.mult, op1=mybir.AluOpType.is_gt,
                    )

                    # pair maxes + predicated select (sel starts as main-diag pm1)
                    sel = post.tile([128, N], F16, tag="sel")
                    nc.vector.tensor_tensor(
                        out=sel[:], in0=Dn[:, 0:N], in1=Up[:, 2 : N + 2],
                        op=mybir.AluOpType.max,
                    )
                    pm3 = post.tile([128, N], F16, tag="pm3")
                    nc.vector.tensor_tensor(
                        out=pm3[:], in0=Dn[:, 2 : N + 2], in1=Up[:, 0:N],
                        op=mybir.AluOpType.max,
                    )
                    pm2 = post.tile([128, N], F16, tag="pm2")
                    nc.vector.tensor_tensor(
                        out=pm2[:], in0=Dn[:, 1 : N + 1], in1=Up[:, 1 : N + 1],
                        op=mybir.AluOpType.max,
                    )
                    pm0 = post.tile([128, N], F16, tag="pm0")
                    nc.vector.tensor_tensor(
                        out=pm0[:], in0=S[:, 0:N], in1=S[:, 2 : N + 2],
                        op=mybir.AluOpType.max,
                    )
                    nc.vector.copy_predicated(sel[:], mneg[:], pm3[:])
                    nc.vector.copy_predicated(sel[:], m2[:], pm2[:])
                    nc.vector.copy_predicated(sel[:], m0[:], pm0[:])

                    # z = zs*S - sel ; out = relu(z) * x
                    z = post.tile([128, N], F16, tag="z")
                    nc.vector.scalar_tensor_tensor(
                        out=z[:], in0=S[:, 1 : N + 1], scalar=zs, in1=sel[:],
                        op0=mybir.AluOpType.mult, op1=mybir.AluOpType.subtract,
                    )
                    o = outp.tile([128, N], F32, tag="o")
                    nc.vector.scalar_tensor_tensor(
                        out=o[:], in0=z[:], scalar=0.0, in1=xh[i][rt][:],
                        op0=mybir.AluOpType.max, op1=mybir.AluOpType.mult,
                    )
                    nc.sync.dma_start(
                        out=out_d[i * N + row0 : i * N + row0 + 128, :], in_=o[:]
                    )

    nc.compile()
    return nc


# ---------------------------------------------------------------------------
# host side
# ---------------------------------------------------------------------------

def _make_band(weights, offsets, pad):
    M = np.zeros((N, N), dtype=np.float64)
    for w, o in zip(weights, offsets):
        idx = np.arange(N)
        src = idx + o
        if pad == "replicate":
            np.add.at(M, (np.clip(src, 0, N - 1), idx), w)
        else:
            ok = (src >= 0) & (src < N)
            np.add.at(M, (src[ok], idx[ok]), w)
    return M


def _host_weights(gauss_kernel):
    gk = np.asarray(gauss_kernel, dtype=np.float64)[0, 0]
    U, sv, Vt = np.linalg.svd(gk)
    assert sv[1] < 1e-5 * sv[0], "gauss kernel not rank-1 separable"
    wv = U[:, 0] * np.sqrt(sv[0])
    wh = Vt[0] * np.sqrt(sv[0])
    if wv.sum() < 0:
        wv, wh = -wv, -wh
    o5 = [-2, -1, 0, 1, 2]
    o3 = [-1, 0, 1]
    Bv = _make_band(wv, o5, "zero")
    Bh = _make_band(wh, o5, "zero")
    Sv = _make_band([1, 2, 1], o3, "replicate")
    Dv = _make_band([-1, 0, 1], o3, "replicate")
    Sh = _make_band([1, 2, 1], o3, "replicate")
    Dh = _make_band([-1, 0, 1], o3, "replicate")
    f16 = lambda a: np.ascontiguousarray(a, dtype=np.float16)
    return {
        "w_bv": f16(Bv),
        "w_bh": f16(Bh),
        "w_mvx": f16(Bv @ Sv),
        "w_mhx": f16(Bh @ Dh),
        "w_mvy": f16(Bv @ Dv),
        "w_mhy": f16(Bh @ Sh),
    }


_NC_CACHE = {}
LAST_RESULT = None


def kernel(reconst, gauss_kernel, nms_kernel):
    nk = np.asarray(nms_kernel, dtype=np.float64)
    cen = float(nk[0, 0, 1, 1])
    v = float(nk[0, 0, 1, 2])
    # verify nms kernel structure: center + single tap v per direction
    pos = [(1, 2), (2, 2), (2, 1), (2, 0), (1, 0), (0, 0), (0, 1), (0, 2)]
    for d, (r, c) in enumerate(pos):
        k = nk[d, 0].copy()
        assert abs(k[1, 1] - cen) < 1e-6 and abs(k[r, c] - v) < 1e-6
        k[1, 1] = 0.0
        k[r, c] = 0.0
        assert np.abs(k).max() < 1e-7
    assert v < 0

    key = (round(cen, 9), round(v, 9))
    if key not in _NC_CACHE:
        _NC_CACHE[key] = build_nc(cen, v)
    nc = _NC_CACHE[key]

    w = _host_weights(gauss_kernel)
    x = np.asarray(reconst, dtype=np.float32).reshape(B_TOTAL, N, N)
    in_maps = []
    for core in range(N_CORES):
        m = {"x": np.ascontiguousarray(
            x[core * B_CORE : (core + 1) * B_CORE].reshape(B_CORE * N, N)
        )}
        m.update(w)
        in_maps.append(m)

    res = run_bass_kernel_spmd(nc, in_maps, core_ids=list(range(N_CORES)))
    global LAST_RESULT
    LAST_RESULT = res
    out = np.concatenate(
        [r["out"].reshape(B_CORE, 1, N, N) for r in res.results], axis=0
    )
    return out.astype(np.float32)


# revision 15
# speedup vs baseline: 1.0600x; 1.0424x over previous
"""Trainium2 Bass kernel for nn_Densenet_with_skip (gauss blur -> sobel ->
angle-binned 8-direction NMS -> gate).

Reformulation (validated vs reference at 1.7e-7 in fp32 numpy):
  b  = gauss5x5(x)                      (zero pad; separable, rank-1)
  gx = (Bv@Sv)^T x (Bh@Dh)              (composed 7-tap bands, replicate pad on b)
  gy = (Bv@Dv)^T x (Bh@Sh)
  bin: m0 = (t1*|gx| >= |gy|)  -> horizontal pair
       m2 = (t1*|gy| >  |gx|)  -> vertical pair
       else diag: gx*gy<0 -> anti-diag pair, else main-diag pair
  out = x * relu(cen*b + v*max(pair))   (pair neighbors, 0 outside image)

All convolutions are done on the TensorEngine as banded matmuls in fp16
(single sweep, ~3.1e-3 rel l2 vs reference). Vertical-pair access is done by
producing row-shifted copies of b (bup/bdn) straight out of the second
(horizontal) matmul pass by shifting the stationary operand's row slice, so
the elementwise stage only ever needs free-dim shifts.
"""

import sys

import numpy as np

sys.path.insert(0, "/opt/trn_rl_repo")

import concourse.bacc as bacc
import concourse.mybir as mybir
from concourse import tile
from concourse.bass_utils import run_bass_kernel_spmd

N = 512
B_TOTAL = 32
N_CORES = 8
B_CORE = B_TOTAL // N_CORES  # 4 images per core
NCHUNK = N // 128  # 4

F16 = mybir.dt.float16
U8 = mybir.dt.uint8
F32 = mybir.dt.float32

T1 = float(np.tan(np.pi / 8))  # tan(22.5 deg)


def _band_ranges(halo):
    """Output-col range [lo, hi) per 128-chunk for a (2*halo+1)-tap band."""
    out = []
    for r in range(NCHUNK):
        lo = max(0, 128 * r - halo)
        hi = min(N, 128 * r + 128 + halo)
        out.append((lo, hi))
    return out


def _banded_mm(nc, psum_ap, lhsT_sl, w_chunks, ranges, out_poff=0):
    """Accumulate sum_r lhsT_r.T @ W_r[:, band] into psum with correct
    PSUM zero-region start/stop handling (each matmul touches either
    all-fresh or all-covered columns)."""
    covered = 0
    n = len(ranges)
    for r in range(n):
        lo, hi = ranges[r]
        first = r == 0
        last = r == n - 1
        if not first and lo < covered:
            nc.tensor.matmul(
                psum_ap[:, lo:covered],
                lhsT_sl[r],
                w_chunks[r][:, lo:covered],
                start=False,
                stop=False,
            )
            lo = covered
        nc.tensor.matmul(
            psum_ap[:, lo:hi],
            lhsT_sl[r],
            w_chunks[r][:, lo:hi],
            start=first,
            stop=last,
        )
        covered = hi


def _banded_mm2(nc, psum_ap, lhsT_a, wa, lhsT_b, wb, ranges):
    covered = 0
    n = len(ranges)
    for r in range(n):
        lo, hi = ranges[r]
        first = r == 0
        if not first and lo < covered:
            nc.tensor.matmul(psum_ap[:, lo:covered], lhsT_a[r],
                             wa[r][:, lo:covered], start=False, stop=False)
            lo = covered
        nc.tensor.matmul(psum_ap[:, lo:hi], lhsT_a[r], wa[r][:, lo:hi],
                         start=first, stop=False)
        covered = hi
    for r in range(n):
        lo, hi = ranges[r]
        last = r == n - 1
        nc.tensor.matmul(psum_ap[:, lo:hi], lhsT_b[r], wb[r][:, lo:hi],
                         start=False, stop=last)


def build_nc(cen, v):
    """Build + compile the per-core program. cen/v: nms kernel center and tap."""
    s = -v  # S = s*b ; z = (cen/s)*S - max(pair of S-neighbors)
    zs = cen / s

    nc = bacc.Bacc("TRN2", target_bir_lowering=False, debug=False)

    x_d = nc.dram_tensor("x", [B_CORE * N, N], F32, kind="ExternalInput").ap()
    w_names = ["bv", "bh", "mvx", "mhx", "mvy", "mhy"]
    w_d = {
        k: nc.dram_tensor("w_" + k, [N, N], F16, kind="ExternalInput").ap()
        for k in w_names
    }
    out_d = nc.dram_tensor("out", [B_CORE * N, N], F32, kind="ExternalOutput").ap()

    r5 = _band_ranges(2)  # 5-tap gauss band
    r7 = _band_ranges(3)  # 7-tap composed sobel band

    with tile.TileContext(nc) as tc:
        with (
            tc.tile_pool(name="wpool", bufs=1) as wpool,
            tc.tile_pool(name="xpool", bufs=1) as xpool,
            tc.tile_pool(name="tT", bufs=2) as tTpool,
            tc.tile_pool(name="post", bufs=2) as post,
            tc.tile_pool(name="outp", bufs=3) as outp,
            tc.tile_pool(name="psumv", bufs=2, space="PSUM") as psumv,
            tc.tile_pool(name="psumh", bufs=1, space="PSUM") as psumh,
        ):
            zrow = wpool.tile([1, N + 2], F16, tag="zrow")
            nc.vector.memset(zrow[:], 0.0)

            # --- load weights (band matrices), 4 row-chunks each ---
            w_sb = {}
            for k in w_names:
                w_sb[k] = []
                for r in range(NCHUNK):
                    t = wpool.tile([128, N], F16, tag=f"w_{k}_{r}")
                    nc.sync.dma_start(out=t[:], in_=w_d[k][128 * r : 128 * (r + 1), :])
                    w_sb[k].append(t)

            # --- load x as fp16 (DMA cast); quad layout [128, 4*512] ---
            xh = []
            for i in range(B_CORE):
                t = xpool.tile([128, NCHUNK * N], F16, tag=f"xh_{i}")
                for r in range(NCHUNK):
                    nc.gpsimd.dma_start(
                        out=t[:, N * r : N * (r + 1)],
                        in_=x_d[i * N + 128 * r : i * N + 128 * (r + 1), :],
                    )
                xh.append(t)

            for i in range(B_CORE):
                # ---------- vertical passes: tT[k][c] = (W_v^T x)^T ----------
                tT = {}
                for k, wname, rr in (
                    ("b", "bv", r5),
                    ("x", "mvx", r7),
                    ("y", "mvy", r7),
                ):
                    tT[k] = []
                    for c in range(NCHUNK):
                        pv = psumv.tile([128, N], F32, tag="pv")
                        lhsT = [xh[i][:, N * r + 128 * c : N * r + 128 * (c + 1)] for r in range(NCHUNK)]
                        _banded_mm(nc, pv, lhsT, w_sb[wname], rr)
                        st = tTpool.tile([128, N], F16, tag=f"tT_{k}_{c}")
                        nc.scalar.activation(
                            st[:], pv[:], mybir.ActivationFunctionType.Copy
                        )
                        tT[k].append(st)

                # ---------- horizontal passes (per row-tile) + quad elementwise ----------
                Sq = post.tile([128, NCHUNK * (N + 2)], F16, tag="S")
                Upq = post.tile([128, NCHUNK * (N + 2)], F16, tag="Up")
                Dnq = post.tile([128, NCHUNK * (N + 2)], F16, tag="Dn")
                t3 = Sq[:].rearrange("p (q w) -> p q w", w=N + 2)
                nc.vector.memset(t3[:, :, 0:1], 0.0)
                nc.vector.memset(t3[:, :, N + 1 : N + 2], 0.0)
                axq = post.tile([128, NCHUNK * N], F16, tag="ax")
                ayq = post.tile([128, NCHUNK * N], F16, tag="ay")
                pabsq = post.tile([128, NCHUNK * N], F16, tag="pabs")

                for rt in range(NCHUNK):
                    row0 = 128 * rt

                    def hpass(key, wname, rr, shift, tag):
                        p = psumh.tile([128, N], F32, tag=tag)
                        lo = row0 + shift
                        lhsT = [tT[key][c][:, lo : lo + 128] for c in range(NCHUNK)]
                        _banded_mm(nc, p, lhsT, w_sb[wname], rr)
                        return p

                    pb = hpass("b", "bh", r5, 0, "pb")
                    pgx = hpass("x", "mhx", r7, 0, "pgx")
                    pgy = hpass("y", "mhy", r7, 0, "pgy")
                    pxy = psumh.tile([128, N], F32, tag="pxy")
                    lo = row0
                    lhsT_x = [tT["x"][c][:, lo : lo + 128] for c in range(NCHUNK)]
                    lhsT_y = [tT["y"][c][:, lo : lo + 128] for c in range(NCHUNK)]
                    _banded_mm2(nc, pxy, lhsT_x, w_sb["mhx"], lhsT_y, w_sb["mhy"], r7)

                    q0 = rt * (N + 2)
                    f0 = rt * N
                    nc.scalar.activation(
                        Sq[:, q0 + 1 : q0 + N + 1], pb[:],
                        mybir.ActivationFunctionType.Copy, scale=s,
                    )
                    nc.scalar.activation(
                        axq[:, f0 : f0 + N], pgx[:],
                        mybir.ActivationFunctionType.Abs,
                    )
                    nc.scalar.activation(
                        ayq[:, f0 : f0 + N], pgy[:],
                        mybir.ActivationFunctionType.Abs,
                    )
                    nc.scalar.activation(
                        pabsq[:, f0 : f0 + N], pxy[:],
                        mybir.ActivationFunctionType.Abs,
                    )

                # Up/Dn = partition-shifted copies of S via DMA (incl pads)
                W2 = N + 2
                S3v = Sq[:].rearrange("p (q w) -> p q w", w=W2)
                U3v = Upq[:].rearrange("p (q w) -> p q w", w=W2)
                D3v = Dnq[:].rearrange("p (q w) -> p q w", w=W2)
                # Up[p, q] = S[p+1, q]; Up[127, q] = S[0, q+1]; Up[127, 3] = 0
                nc.sync.dma_start(out=U3v[0:127, :, :], in_=S3v[1:128, :, :])
                nc.sync.dma_start(out=U3v[127:128, 0:3, :], in_=S3v[0:1, 1:4, :])
                nc.sync.dma_start(
                    out=U3v[127:128, 3:4, :],
                    in_=zrow[:].rearrange("p (q w) -> p q w", w=W2),
                )
                # Dn[p, q] = S[p-1, q]; Dn[0, q] = S[127, q-1]; Dn[0, 0] = 0
                nc.sync.dma_start(out=D3v[1:128, :, :], in_=S3v[0:127, :, :])
                nc.sync.dma_start(out=D3v[0:1, 1:4, :], in_=S3v[127:128, 0:3, :])
                nc.sync.dma_start(
                    out=D3v[0:1, 0:1, :],
                    in_=zrow[:].rearrange("p (q w) -> p q w", w=W2),
                )
                S3 = Sq[:].rearrange("p (q w) -> p q w", w=W2)
                U3 = Upq[:].rearrange("p (q w) -> p q w", w=W2)
                D3 = Dnq[:].rearrange("p (q w) -> p q w", w=W2)
                ax3 = axq[:].rearrange("p (q w) -> p q w", w=N)
                ay3 = ayq[:].rearrange("p (q w) -> p q w", w=N)
                pa3 = pabsq[:].rearrange("p (q w) -> p q w", w=N)

                def qt(tag, dt=F16):
                    t = post.tile([128, NCHUNK * N], dt, tag=tag)
                    return t, t[:].rearrange("p (q w) -> p q w", w=N)

                selq, sel3 = qt("sel")
                pm3q, pm33 = qt("pm3")
                pm2q, pm23 = qt("pm2")
                pm0q, pm03 = qt("pm0")
                mmaxq, mmax3 = qt("mmax")
                mnegq, mneg3 = qt("mneg", U8)
                m0q, m03 = qt("m0", U8)
                m2q, m23 = qt("m2", U8)
                zq, z3 = qt("z")

                TT = nc.vector.tensor_tensor
                TT(out=sel3, in0=D3[:, :, 0:N], in1=U3[:, :, 2 : N + 2],
                   op=mybir.AluOpType.max)
                TT(out=pm33, in0=D3[:, :, 2 : N + 2], in1=U3[:, :, 0:N],
                   op=mybir.AluOpType.max)
                TT(out=pm23, in0=D3[:, :, 1 : N + 1], in1=U3[:, :, 1 : N + 1],
                   op=mybir.AluOpType.max)
                TT(out=pm03, in0=S3[:, :, 0:N], in1=S3[:, :, 2 : N + 2],
                   op=mybir.AluOpType.max)
                TT(out=mmax3, in0=ax3, in1=ay3, op=mybir.AluOpType.max)
                TT(out=mneg3, in0=pa3, in1=mmax3, op=mybir.AluOpType.is_lt)
                nc.vector.scalar_tensor_tensor(
                    out=m03, in0=ax3, scalar=T1, in1=ay3,
                    op0=mybir.AluOpType.mult, op1=mybir.AluOpType.is_ge,
                )
                nc.vector.scalar_tensor_tensor(
                    out=m23, in0=ay3, scalar=T1, in1=ax3,
                    op0=mybir.AluOpType.mult, op1=mybir.AluOpType.is_gt,
                )
                nc.vector.copy_predicated(sel3, mneg3, pm33)
                nc.vector.copy_predicated(sel3, m23, pm23)
                nc.vector.copy_predicated(sel3, m03, pm03)
                nc.vector.scalar_tensor_tensor(
                    out=z3, in0=S3[:, :, 1 : N + 1], scalar=zs, in1=sel3,
                    op0=mybir.AluOpType.mult, op1=mybir.AluOpType.subtract,
                )
                oq = outp.tile([128, NCHUNK * N], F16, tag="o")
                o3 = oq[:].rearrange("p (q w) -> p q w", w=N)
                x3 = xh[i][:].rearrange("p (q w) -> p q w", w=N)
                nc.vector.scalar_tensor_tensor(
                    out=o3, in0=z3, scalar=0.0, in1=x3,
                    op0=mybir.AluOpType.max, op1=mybir.AluOpType.mult,
                )
                for rt in range(NCHUNK):
                    nc.gpsimd.dma_start(
                        out=out_d[i * N + 128 * rt : i * N + 128 * (rt + 1), :],
                        in_=oq[:, N * rt : N * (rt + 1)],
                    )

    nc.compile()
    return nc


# ---------------------------------------------------------------------------
# host side
# ---------------------------------------------------------------------------

def _make_band(weights, offsets, pad):
    M = np.zeros((N, N), dtype=np.float64)
    for w, o in zip(weights, offsets):
        idx = np.arange(N)
        src = idx + o
        if pad == "replicate":
            np.add.at(M, (np.clip(src, 0, N - 1), idx), w)
        else:
            ok = (src >= 0) & (src < N)
            np.add.at(M, (src[ok], idx[ok]), w)
    return M


def _host_weights(gauss_kernel):
    gk = np.asarray(gauss_kernel, dtype=np.float64)[0, 0]
    U, sv, Vt = np.linalg.svd(gk)
    assert sv[1] < 1e-5 * sv[0], "gauss kernel not rank-1 separable"
    wv = U[:, 0] * np.sqrt(sv[0])
    wh = Vt[0] * np.sqrt(sv[0])
    if wv.sum() < 0:
        wv, wh = -wv, -wh
    o5 = [-2, -1, 0, 1, 2]
    o3 = [-1, 0, 1]
    Bv = _make_band(wv, o5, "zero")
    Bh = _make_band(wh, o5, "zero")
    Sv = _make_band([1, 2, 1], o3, "replicate")
    Dv = _make_band([-1, 0, 1], o3, "replicate")
    Sh = _make_band([1, 2, 1], o3, "replicate")
    Dh = _make_band([-1, 0, 1], o3, "replicate")
    f16 = lambda a: np.ascontiguousarray(a, dtype=np.float16)
    return {
        "w_bv": f16(Bv),
        "w_bh": f16(Bh),
        "w_mvx": f16(Bv @ Sv),
        "w_mhx": f16(Bh @ Dh),
        "w_mvy": f16(Bv @ Dv),
        "w_mhy": f16(Bh @ Sh),
    }


_NC_CACHE = {}
LAST_RESULT = None


def kernel(reconst, gauss_kernel, nms_kernel):
    nk = np.asarray(nms_kernel, dtype=np.float64)
    cen = float(nk[0, 0, 1, 1])
    v = float(nk[0, 0, 1, 2])
    # verify nms kernel structure: center + single tap v per direction
    pos = [(1, 2), (2, 2), (2, 1), (2, 0), (1, 0), (0, 0), (0, 1), (0, 2)]
    for d, (r, c) in enumerate(pos):
        k = nk[d, 0].copy()
        assert abs(k[1, 1] - cen) < 1e-6 and abs(k[r, c] - v) < 1e-6
        k[1, 1] = 0.0
        k[r, c] = 0.0
        assert np.abs(k).max() < 1e-7
    assert v < 0

    key = (round(cen, 9), round(v, 9))
    if key not in _NC_CACHE:
        _NC_CACHE[key] = build_nc(cen, v)
    nc = _NC_CACHE[key]

    w = _host_weights(gauss_kernel)
    x = np.asarray(reconst, dtype=np.float32).reshape(B_TOTAL, N, N)
    in_maps = []
    for core in range(N_CORES):
        m = {"x": np.ascontiguousarray(
            x[core * B_CORE : (core + 1) * B_CORE].reshape(B_CORE * N, N)
        )}
        m.update(w)
        in_maps.append(m)

    res = run_bass_kernel_spmd(nc, in_maps, core_ids=list(range(N_CORES)))
    global LAST_RESULT
    LAST_RESULT = res
    out = np.concatenate(
        [r["out"].reshape(B_CORE, 1, N, N) for r in res.results], axis=0
    )
    return out.astype(np.float32)


# revision 16
# speedup vs baseline: 20480.7821x; 19322.2505x over previous
"""Trainium2 Bass kernel for nn_Densenet_with_skip (gauss blur -> sobel ->
angle-binned 8-direction NMS -> gate).

Reformulation (validated vs reference at 1.7e-7 in fp32 numpy):
  b  = gauss5x5(x)                      (zero pad; separable, rank-1)
  gx = (Bv@Sv)^T x (Bh@Dh)              (composed 7-tap bands, replicate pad on b)
  gy = (Bv@Dv)^T x (Bh@Sh)
  bin: m0 = (t1*|gx| >= |gy|)  -> horizontal pair
       m2 = (t1*|gy| >  |gx|)  -> vertical pair
       else diag: gx*gy<0 -> anti-diag pair, else main-diag pair
  out = x * relu(cen*b + v*max(pair))   (pair neighbors, 0 outside image)

All convolutions are done on the TensorEngine as banded matmuls in fp16
(single sweep, ~3.1e-3 rel l2 vs reference). Vertical-pair access is done by
producing row-shifted copies of b (bup/bdn) straight out of the second
(horizontal) matmul pass by shifting the stationary operand's row slice, so
the elementwise stage only ever needs free-dim shifts.
"""

import sys

import numpy as np

sys.path.insert(0, "/opt/trn_rl_repo")

import concourse.bacc as bacc
import concourse.mybir as mybir
from concourse import tile
from concourse.bass_utils import run_bass_kernel_spmd

N = 512
B_TOTAL = 32
N_CORES = 8
B_CORE = B_TOTAL // N_CORES  # 4 images per core
NCHUNK = N // 128  # 4

F16 = mybir.dt.float16
U8 = mybir.dt.uint8
F32 = mybir.dt.float32

T1 = float(np.tan(np.pi / 8))  # tan(22.5 deg)


def _band_ranges(halo):
    """Output-col range [lo, hi) per 128-chunk for a (2*halo+1)-tap band."""
    out = []
    for r in range(NCHUNK):
        lo = max(0, 128 * r - halo)
        hi = min(N, 128 * r + 128 + halo)
        out.append((lo, hi))
    return out


def _banded_mm(nc, psum_ap, lhsT_sl, w_chunks, ranges, out_poff=0):
    """Accumulate sum_r lhsT_r.T @ W_r[:, band] into psum with correct
    PSUM zero-region start/stop handling (each matmul touches either
    all-fresh or all-covered columns)."""
    covered = 0
    n = len(ranges)
    for r in range(n):
        lo, hi = ranges[r]
        first = r == 0
        last = r == n - 1
        if not first and lo < covered:
            nc.tensor.matmul(
                psum_ap[:, lo:covered],
                lhsT_sl[r],
                w_chunks[r][:, lo:covered],
                start=False,
                stop=False,
            )
            lo = covered
        nc.tensor.matmul(
            psum_ap[:, lo:hi],
            lhsT_sl[r],
            w_chunks[r][:, lo:hi],
            start=first,
            stop=last,
        )
        covered = hi


def _banded_mm2(nc, psum_ap, lhsT_a, wa, lhsT_b, wb, ranges):
    covered = 0
    n = len(ranges)
    for r in range(n):
        lo, hi = ranges[r]
        first = r == 0
        if not first and lo < covered:
            nc.tensor.matmul(psum_ap[:, lo:covered], lhsT_a[r],
                             wa[r][:, lo:covered], start=False, stop=False)
            lo = covered
        nc.tensor.matmul(psum_ap[:, lo:hi], lhsT_a[r], wa[r][:, lo:hi],
                         start=first, stop=False)
        covered = hi
    for r in range(n):
        lo, hi = ranges[r]
        last = r == n - 1
        nc.tensor.matmul(psum_ap[:, lo:hi], lhsT_b[r], wb[r][:, lo:hi],
                         start=False, stop=last)


def build_nc(cen, v):
    """Build + compile the per-core program. cen/v: nms kernel center and tap."""
    s = -v  # S = s*b ; z = (cen/s)*S - max(pair of S-neighbors)
    zs = cen / s

    nc = bacc.Bacc("TRN2", target_bir_lowering=False, debug=False)

    x_d = nc.dram_tensor("x", [B_CORE * N, N], F32, kind="ExternalInput").ap()
    w_names = ["bv", "bh", "mvx", "mhx", "mvy", "mhy"]
    w_d = {
        k: nc.dram_tensor("w_" + k, [N, N], F16, kind="ExternalInput").ap()
        for k in w_names
    }
    out_d = nc.dram_tensor("out", [B_CORE * N, N], F32, kind="ExternalOutput").ap()

    r5 = _band_ranges(2)  # 5-tap gauss band
    r7 = _band_ranges(3)  # 7-tap composed sobel band

    with tile.TileContext(nc) as tc:
        with (
            tc.tile_pool(name="wpool", bufs=1) as wpool,
            tc.tile_pool(name="xpool", bufs=1) as xpool,
            tc.tile_pool(name="tT", bufs=2) as tTpool,
            tc.tile_pool(name="post", bufs=2) as post,
            tc.tile_pool(name="outp", bufs=3) as outp,
            tc.tile_pool(name="psumv", bufs=2, space="PSUM") as psumv,
            tc.tile_pool(name="psumh", bufs=1, space="PSUM") as psumh,
        ):
            zrow = wpool.tile([1, N + 2], F16, tag="zrow")
            nc.vector.memset(zrow[:], 0.0)

            # --- load weights (band matrices), 4 row-chunks each ---
            w_sb = {}
            for k in w_names:
                w_sb[k] = []
                for r in range(NCHUNK):
                    t = wpool.tile([128, N], F16, tag=f"w_{k}_{r}")
                    nc.sync.dma_start(out=t[:], in_=w_d[k][128 * r : 128 * (r + 1), :])
                    w_sb[k].append(t)

            # --- load x as fp16 (DMA cast); quad layout [128, 4*512] ---
            xh = []
            for i in range(B_CORE):
                t = xpool.tile([128, NCHUNK * N], F16, tag=f"xh_{i}")
                for r in range(NCHUNK):
                    nc.gpsimd.dma_start(
                        out=t[:, N * r : N * (r + 1)],
                        in_=x_d[i * N + 128 * r : i * N + 128 * (r + 1), :],
                    )
                xh.append(t)

            for i in range(B_CORE):
                # ---------- vertical passes: tT[k][c] = (W_v^T x)^T ----------
                tT = {}
                for k, wname, rr in (
                    ("b", "bv", r5),
                    ("x", "mvx", r7),
                    ("y", "mvy", r7),
                ):
                    tT[k] = []
                    for c in range(NCHUNK):
                        pv = psumv.tile([128, N], F32, tag="pv")
                        lhsT = [xh[i][:, N * r + 128 * c : N * r + 128 * (c + 1)] for r in range(NCHUNK)]
                        _banded_mm(nc, pv, lhsT, w_sb[wname], rr)
                        st = tTpool.tile([128, N], F16, tag=f"tT_{k}_{c}")
                        nc.scalar.activation(
                            st[:], pv[:], mybir.ActivationFunctionType.Copy
                        )
                        tT[k].append(st)

                # ---------- horizontal passes (per row-tile) + quad elementwise ----------
                Sq = post.tile([128, NCHUNK * (N + 2)], F16, tag="S")
                Upq = post.tile([128, NCHUNK * (N + 2)], F16, tag="Up")
                Dnq = post.tile([128, NCHUNK * (N + 2)], F16, tag="Dn")
                t3 = Sq[:].rearrange("p (q w) -> p q w", w=N + 2)
                nc.vector.memset(t3[:, :, 0:1], 0.0)
                nc.vector.memset(t3[:, :, N + 1 : N + 2], 0.0)
                axq = post.tile([128, NCHUNK * N], F16, tag="ax")
                ayq = post.tile([128, NCHUNK * N], F16, tag="ay")
                pabsq = post.tile([128, NCHUNK * N], F16, tag="pabs")

                for rt in range(NCHUNK):
                    row0 = 128 * rt

                    def hpass(key, wname, rr, shift, tag):
                        p = psumh.tile([128, N], F32, tag=tag)
                        lo = row0 + shift
                        lhsT = [tT[key][c][:, lo : lo + 128] for c in range(NCHUNK)]
                        _banded_mm(nc, p, lhsT, w_sb[wname], rr)
                        return p

                    pb = hpass("b", "bh", r5, 0, "pb")
                    pgx = hpass("x", "mhx", r7, 0, "pgx")
                    pgy = hpass("y", "mhy", r7, 0, "pgy")
                    pxy = psumh.tile([128, N], F32, tag="pxy")
                    lo = row0
                    lhsT_x = [tT["x"][c][:, lo : lo + 128] for c in range(NCHUNK)]
                    lhsT_y = [tT["y"][c][:, lo : lo + 128] for c in range(NCHUNK)]
                    _banded_mm2(nc, pxy, lhsT_x, w_sb["mhx"], lhsT_y, w_sb["mhy"], r7)

                    q0 = rt * (N + 2)
                    f0 = rt * N
                    nc.scalar.activation(
                        Sq[:, q0 + 1 : q0 + N + 1], pb[:],
                        mybir.ActivationFunctionType.Copy, scale=s,
                    )
                    nc.scalar.activation(
                        axq[:, f0 : f0 + N], pgx[:],
                        mybir.ActivationFunctionType.Abs,
                    )
                    nc.scalar.activation(
                        ayq[:, f0 : f0 + N], pgy[:],
                        mybir.ActivationFunctionType.Abs,
                    )
                    nc.scalar.activation(
                        pabsq[:, f0 : f0 + N], pxy[:],
                        mybir.ActivationFunctionType.Abs,
                    )

                # Up/Dn = partition-shifted copies of S via DMA (incl pads)
                W2 = N + 2
                S3v = Sq[:].rearrange("p (q w) -> p q w", w=W2)
                U3v = Upq[:].rearrange("p (q w) -> p q w", w=W2)
                D3v = Dnq[:].rearrange("p (q w) -> p q w", w=W2)
                # Up[p, q] = S[p+1, q]; Up[127, q] = S[0, q+1]; Up[127, 3] = 0
                nc.sync.dma_start(out=U3v[0:127, :, :], in_=S3v[1:128, :, :])
                nc.sync.dma_start(out=U3v[127:128, 0:3, :], in_=S3v[0:1, 1:4, :])
                nc.sync.dma_start(
                    out=U3v[127:128, 3:4, :],
                    in_=zrow[:].rearrange("p (q w) -> p q w", w=W2),
                )
                # Dn[p, q] = S[p-1, q]; Dn[0, q] = S[127, q-1]; Dn[0, 0] = 0
                nc.sync.dma_start(out=D3v[1:128, :, :], in_=S3v[0:127, :, :])
                nc.sync.dma_start(out=D3v[0:1, 1:4, :], in_=S3v[127:128, 0:3, :])
                nc.sync.dma_start(
                    out=D3v[0:1, 0:1, :],
                    in_=zrow[:].rearrange("p (q w) -> p q w", w=W2),
                )
                S3 = Sq[:].rearrange("p (q w) -> p q w", w=W2)
                U3 = Upq[:].rearrange("p (q w) -> p q w", w=W2)
                D3 = Dnq[:].rearrange("p (q w) -> p q w", w=W2)
                ax3 = axq[:].rearrange("p (q w) -> p q w", w=N)
                ay3 = ayq[:].rearrange("p (q w) -> p q w", w=N)
                pa3 = pabsq[:].rearrange("p (q w) -> p q w", w=N)

                def qt(tag, dt=F16):
                    t = post.tile([128, NCHUNK * N], dt, tag=tag)
                    return t, t[:].rearrange("p (q w) -> p q w", w=N)

                selq, sel3 = qt("sel")
                pm3q, pm33 = qt("pm3")
                pm2q, pm23 = qt("pm2")
                pm0q, pm03 = qt("pm0")
                mnegq, mneg3 = qt("mneg", U8)
                m0q, m03 = qt("m0", U8)
                m2q, m23 = qt("m2", U8)
                zq, z3 = qt("z")

                TT = nc.vector.tensor_tensor
                TT(out=sel3, in0=D3[:, :, 0:N], in1=U3[:, :, 2 : N + 2],
                   op=mybir.AluOpType.max)
                TT(out=pm33, in0=D3[:, :, 2 : N + 2], in1=U3[:, :, 0:N],
                   op=mybir.AluOpType.max)
                TT(out=pm23, in0=D3[:, :, 1 : N + 1], in1=U3[:, :, 1 : N + 1],
                   op=mybir.AluOpType.max)
                TT(out=pm03, in0=S3[:, :, 0:N], in1=S3[:, :, 2 : N + 2],
                   op=mybir.AluOpType.max)
                nc.vector.scalar_tensor_tensor(
                    out=mneg3, in0=ax3, scalar=float(np.sqrt(2.0)), in1=pa3,
                    op0=mybir.AluOpType.mult, op1=mybir.AluOpType.is_gt,
                )
                nc.vector.scalar_tensor_tensor(
                    out=m03, in0=ax3, scalar=T1, in1=ay3,
                    op0=mybir.AluOpType.mult, op1=mybir.AluOpType.is_ge,
                )
                nc.vector.scalar_tensor_tensor(
                    out=m23, in0=ay3, scalar=T1, in1=ax3,
                    op0=mybir.AluOpType.mult, op1=mybir.AluOpType.is_gt,
                )
                nc.vector.copy_predicated(sel3, mneg3, pm33)
                nc.vector.copy_predicated(sel3, m23, pm23)
                nc.vector.copy_predicated(sel3, m03, pm03)
                nc.vector.scalar_tensor_tensor(
                    out=z3, in0=S3[:, :, 1 : N + 1], scalar=zs, in1=sel3,
                    op0=mybir.AluOpType.mult, op1=mybir.AluOpType.subtract,
                )
                oq = outp.tile([128, NCHUNK * N], F16, tag="o")
                o3 = oq[:].rearrange("p (q w) -> p q w", w=N)
                x3 = xh[i][:].rearrange("p (q w) -> p q w", w=N)
                nc.vector.scalar_tensor_tensor(
                    out=o3, in0=z3, scalar=0.0, in1=x3,
                    op0=mybir.AluOpType.max, op1=mybir.AluOpType.mult,
                )
                for rt in range(NCHUNK):
                    nc.gpsimd.dma_start(
                        out=out_d[i * N + 128 * rt : i * N + 128 * (rt + 1), :],
                        in_=oq[:, N * rt : N * (rt + 1)],
                    )

    nc.compile()
    return nc


# ---------------------------------------------------------------------------
# host side
# ---------------------------------------------------------------------------

def _make_band(weights, offsets, pad):
    M = np.zeros((N, N), dtype=np.float64)
    for w, o in zip(weights, offsets):
        idx = np.arange(N)
        src = idx + o
        if pad == "replicate":
            np.add.at(M, (np.clip(src, 0, N - 1), idx), w)
        else:
            ok = (src >= 0) & (src < N)
            np.add.at(M, (src[ok], idx[ok]), w)
    return M


def _host_weights(gauss_kernel):
    gk = np.asarray(gauss_kernel, dtype=np.float64)[0, 0]
    U, sv, Vt = np.linalg.svd(gk)
    assert sv[1] < 1e-5 * sv[0], "gauss kernel not rank-1 separable"
    wv = U[:, 0] * np.sqrt(sv[0])
    wh = Vt[0] * np.sqrt(sv[0])
    if wv.sum() < 0:
        wv, wh = -wv, -wh
    o5 = [-2, -1, 0, 1, 2]
    o3 = [-1, 0, 1]
    Bv = _make_band(wv, o5, "zero")
    Bh = _make_band(wh, o5, "zero")
    Sv = _make_band([1, 2, 1], o3, "replicate")
    Dv = _make_band([-1, 0, 1], o3, "replicate")
    Sh = _make_band([1, 2, 1], o3, "replicate")
    Dh = _make_band([-1, 0, 1], o3, "replicate")
    f16 = lambda a: np.ascontiguousarray(a, dtype=np.float16)
    return {
        "w_bv": f16(Bv),
        "w_bh": f16(Bh),
        "w_mvx": f16(Bv @ Sv),
        "w_mhx": f16(Bh @ Dh),
        "w_mvy": f16(Bv @ Dv),
        "w_mhy": f16(Bh @ Sh),
    }


_NC_CACHE = {}
LAST_RESULT = None


def kernel(reconst, gauss_kernel, nms_kernel):
    nk = np.asarray(nms_kernel, dtype=np.float64)
    cen = float(nk[0, 0, 1, 1])
    v = float(nk[0, 0, 1, 2])
    # verify nms kernel structure: center + single tap v per direction
    pos = [(1, 2), (2, 2), (2, 1), (2, 0), (1, 0), (0, 0), (0, 1), (0, 2)]
    for d, (r, c) in enumerate(pos):
        k = nk[d, 0].copy()
        assert abs(k[1, 1] - cen) < 1e-6 and abs(k[r, c] - v) < 1e-6
        k[1, 1] = 0.0
        k[r, c] = 0.0
        assert np.abs(k).max() < 1e-7
    assert v < 0

    key = (round(cen, 9), round(v, 9))
    if key not in _NC_CACHE:
        _NC_CACHE[key] = build_nc(cen, v)
    nc = _NC_CACHE[key]

    w = _host_weights(gauss_kernel)
    x = np.asarray(reconst, dtype=np.float32).reshape(B_TOTAL, N, N)
    in_maps = []
    for core in range(N_CORES):
        m = {"x": np.ascontiguousarray(
            x[core * B_CORE : (core + 1) * B_CORE].reshape(B_CORE * N, N)
        )}
        m.update(w)
        in_maps.append(m)

    res = run_bass_kernel_spmd(nc, in_maps, core_ids=list(range(N_CORES)))
    global LAST_RESULT
    LAST_RESULT = res
    out = np.concatenate(
        [r["out"].reshape(B_CORE, 1, N, N) for r in res.results], axis=0
    )
    return out.astype(np.float32)


# revision 17
# speedup vs baseline: 20579.3255x; 1.0048x over previous
"""Trainium2 Bass kernel for nn_Densenet_with_skip (gauss blur -> sobel ->
angle-binned 8-direction NMS -> gate).

Reformulation (validated vs reference at 1.7e-7 in fp32 numpy):
  b  = gauss5x5(x)                      (zero pad; separable, rank-1)
  gx = (Bv@Sv)^T x (Bh@Dh)              (composed 7-tap bands, replicate pad on b)
  gy = (Bv@Dv)^T x (Bh@Sh)
  bin: m0 = (t1*|gx| >= |gy|)  -> horizontal pair
       m2 = (t1*|gy| >  |gx|)  -> vertical pair
       else diag: gx*gy<0 -> anti-diag pair, else main-diag pair
  out = x * relu(cen*b + v*max(pair))   (pair neighbors, 0 outside image)

All convolutions are done on the TensorEngine as banded matmuls in fp16
(single sweep, ~3.1e-3 rel l2 vs reference). Vertical-pair access is done by
producing row-shifted copies of b (bup/bdn) straight out of the second
(horizontal) matmul pass by shifting the stationary operand's row slice, so
the elementwise stage only ever needs free-dim shifts.
"""

import sys

import numpy as np

sys.path.insert(0, "/opt/trn_rl_repo")

import concourse.bacc as bacc
import concourse.mybir as mybir
from concourse import tile
from concourse.bass_utils import run_bass_kernel_spmd

N = 512
B_TOTAL = 32
N_CORES = 8
B_CORE = B_TOTAL // N_CORES  # 4 images per core
NCHUNK = N // 128  # 4

F16 = mybir.dt.float16
U8 = mybir.dt.uint8
F32 = mybir.dt.float32

T1 = float(np.tan(np.pi / 8))  # tan(22.5 deg)


def _band_ranges(halo):
    """Output-col range [lo, hi) per 128-chunk for a (2*halo+1)-tap band."""
    out = []
    for r in range(NCHUNK):
        lo = max(0, 128 * r - halo)
        hi = min(N, 128 * r + 128 + halo)
        out.append((lo, hi))
    return out


def _banded_mm(nc, psum_ap, lhsT_sl, w_chunks, ranges, out_poff=0):
    """Accumulate sum_r lhsT_r.T @ W_r[:, band] into psum with correct
    PSUM zero-region start/stop handling (each matmul touches either
    all-fresh or all-covered columns)."""
    covered = 0
    n = len(ranges)
    for r in range(n):
        lo, hi = ranges[r]
        first = r == 0
        last = r == n - 1
        if not first and lo < covered:
            nc.tensor.matmul(
                psum_ap[:, lo:covered],
                lhsT_sl[r],
                w_chunks[r][:, lo:covered],
                start=False,
                stop=False,
            )
            lo = covered
        nc.tensor.matmul(
            psum_ap[:, lo:hi],
            lhsT_sl[r],
            w_chunks[r][:, lo:hi],
            start=first,
            stop=last,
        )
        covered = hi


def _banded_mm2(nc, psum_ap, lhsT_a, wa, lhsT_b, wb, ranges):
    covered = 0
    n = len(ranges)
    for r in range(n):
        lo, hi = ranges[r]
        first = r == 0
        if not first and lo < covered:
            nc.tensor.matmul(psum_ap[:, lo:covered], lhsT_a[r],
                             wa[r][:, lo:covered], start=False, stop=False)
            lo = covered
        nc.tensor.matmul(psum_ap[:, lo:hi], lhsT_a[r], wa[r][:, lo:hi],
                         start=first, stop=False)
        covered = hi
    for r in range(n):
        lo, hi = ranges[r]
        last = r == n - 1
        nc.tensor.matmul(psum_ap[:, lo:hi], lhsT_b[r], wb[r][:, lo:hi],
                         start=False, stop=last)


def build_nc(cen, v):
    """Build + compile the per-core program. cen/v: nms kernel center and tap."""
    s = -v  # S = s*b ; z = (cen/s)*S - max(pair of S-neighbors)
    zs = cen / s

    nc = bacc.Bacc("TRN2", target_bir_lowering=False, debug=False)

    x_d = nc.dram_tensor("x", [B_CORE * N, N], F32, kind="ExternalInput").ap()
    w_names = ["bv", "bh", "mvx", "mhx", "mvy", "mhy"]
    w_d = {
        k: nc.dram_tensor("w_" + k, [N, N], F16, kind="ExternalInput").ap()
        for k in w_names
    }
    out_d = nc.dram_tensor("out", [B_CORE * N, N], F32, kind="ExternalOutput").ap()

    r5 = _band_ranges(2)  # 5-tap gauss band
    r7 = _band_ranges(3)  # 7-tap composed sobel band

    with tile.TileContext(nc) as tc:
        with (
            tc.tile_pool(name="wpool", bufs=1) as wpool,
            tc.tile_pool(name="xpool", bufs=1) as xpool,
            tc.tile_pool(name="tT", bufs=2) as tTpool,
            tc.tile_pool(name="post", bufs=2) as post,
            tc.tile_pool(name="outp", bufs=3) as outp,
            tc.tile_pool(name="psumv", bufs=2, space="PSUM") as psumv,
            tc.tile_pool(name="psumh", bufs=2, space="PSUM") as psumh,
        ):
            zrow = wpool.tile([1, N + 2], F16, tag="zrow")
            nc.vector.memset(zrow[:], 0.0)

            # --- load weights (band matrices), 4 row-chunks each ---
            w_sb = {}
            for k in w_names:
                w_sb[k] = []
                for r in range(NCHUNK):
                    t = wpool.tile([128, N], F16, tag=f"w_{k}_{r}")
                    nc.sync.dma_start(out=t[:], in_=w_d[k][128 * r : 128 * (r + 1), :])
                    w_sb[k].append(t)

            # --- load x as fp16 (DMA cast); quad layout [128, 4*512] ---
            xh = []
            for i in range(B_CORE):
                t = xpool.tile([128, NCHUNK * N], F16, tag=f"xh_{i}")
                for r in range(NCHUNK):
                    nc.gpsimd.dma_start(
                        out=t[:, N * r : N * (r + 1)],
                        in_=x_d[i * N + 128 * r : i * N + 128 * (r + 1), :],
                    )
                xh.append(t)

            for i in range(B_CORE):
                # ---------- vertical passes: tT[k][c] = (W_v^T x)^T ----------
                tT = {}
                for k, wname, rr in (
                    ("b", "bv", r5),
                    ("x", "mvx", r7),
                    ("y", "mvy", r7),
                ):
                    tT[k] = []
                    for c in range(NCHUNK):
                        pv = psumv.tile([128, N], F32, tag="pv")
                        lhsT = [xh[i][:, N * r + 128 * c : N * r + 128 * (c + 1)] for r in range(NCHUNK)]
                        _banded_mm(nc, pv, lhsT, w_sb[wname], rr)
                        st = tTpool.tile([128, N], F16, tag=f"tT_{k}_{c}")
                        nc.scalar.activation(
                            st[:], pv[:], mybir.ActivationFunctionType.Copy
                        )
                        tT[k].append(st)

                # ---------- horizontal passes (per row-tile) + quad elementwise ----------
                Sq = post.tile([128, NCHUNK * (N + 2)], F16, tag="S")
                Upq = post.tile([128, NCHUNK * (N + 2)], F16, tag="Up")
                Dnq = post.tile([128, NCHUNK * (N + 2)], F16, tag="Dn")
                t3 = Sq[:].rearrange("p (q w) -> p q w", w=N + 2)
                nc.vector.memset(t3[:, :, 0:1], 0.0)
                nc.vector.memset(t3[:, :, N + 1 : N + 2], 0.0)
                axq = post.tile([128, NCHUNK * N], F16, tag="ax")
                ayq = post.tile([128, NCHUNK * N], F16, tag="ay")
                pabsq = post.tile([128, NCHUNK * N], F16, tag="pabs")

                for rt in range(NCHUNK):
                    row0 = 128 * rt

                    def hpass(key, wname, rr, shift, tag):
                        p = psumh.tile([128, N], F32, tag=tag)
                        lo = row0 + shift
                        lhsT = [tT[key][c][:, lo : lo + 128] for c in range(NCHUNK)]
                        _banded_mm(nc, p, lhsT, w_sb[wname], rr)
                        return p

                    pb = hpass("b", "bh", r5, 0, "pb")
                    pgx = hpass("x", "mhx", r7, 0, "pgx")
                    # pgy holds gy; its accumulation group stays open. After
                    # the |gy| extract, one extra sweep adds gx into the same
                    # psum (sign test: sign(gx*gy)<0 iff |gx+gy| < sqrt2*|gx|).
                    pgy = psumh.tile([128, N], F32, tag="pgy")
                    lhsT_y = [tT["y"][c][:, row0 : row0 + 128] for c in range(NCHUNK)]
                    covered = 0
                    for r in range(NCHUNK):
                        lo2, hi2 = r7[r]
                        first = r == 0
                        if not first and lo2 < covered:
                            nc.tensor.matmul(pgy[:, lo2:covered], lhsT_y[r],
                                             w_sb["mhy"][r][:, lo2:covered],
                                             start=False, stop=False)
                            lo2 = covered
                        nc.tensor.matmul(pgy[:, lo2:hi2], lhsT_y[r],
                                         w_sb["mhy"][r][:, lo2:hi2],
                                         start=first, stop=False)
                        covered = hi2

                    q0 = rt * (N + 2)
                    f0 = rt * N
                    nc.scalar.activation(
                        Sq[:, q0 + 1 : q0 + N + 1], pb[:],
                        mybir.ActivationFunctionType.Copy, scale=s,
                    )
                    nc.scalar.activation(
                        axq[:, f0 : f0 + N], pgx[:],
                        mybir.ActivationFunctionType.Abs,
                    )
                    nc.scalar.activation(
                        ayq[:, f0 : f0 + N], pgy[:],
                        mybir.ActivationFunctionType.Abs,
                    )
                    lhsT_x = [tT["x"][c][:, row0 : row0 + 128] for c in range(NCHUNK)]
                    for r in range(NCHUNK):
                        lo2, hi2 = r7[r]
                        nc.tensor.matmul(pgy[:, lo2:hi2], lhsT_x[r],
                                         w_sb["mhx"][r][:, lo2:hi2],
                                         start=False, stop=(r == NCHUNK - 1))
                    nc.scalar.activation(
                        pabsq[:, f0 : f0 + N], pgy[:],
                        mybir.ActivationFunctionType.Abs,
                    )

                # Up/Dn = partition-shifted copies of S via DMA (incl pads)
                W2 = N + 2
                S3v = Sq[:].rearrange("p (q w) -> p q w", w=W2)
                U3v = Upq[:].rearrange("p (q w) -> p q w", w=W2)
                D3v = Dnq[:].rearrange("p (q w) -> p q w", w=W2)
                # Up[p, q] = S[p+1, q]; Up[127, q] = S[0, q+1]; Up[127, 3] = 0
                nc.sync.dma_start(out=U3v[0:127, :, :], in_=S3v[1:128, :, :])
                nc.sync.dma_start(out=U3v[127:128, 0:3, :], in_=S3v[0:1, 1:4, :])
                nc.sync.dma_start(
                    out=U3v[127:128, 3:4, :],
                    in_=zrow[:].rearrange("p (q w) -> p q w", w=W2),
                )
                # Dn[p, q] = S[p-1, q]; Dn[0, q] = S[127, q-1]; Dn[0, 0] = 0
                nc.sync.dma_start(out=D3v[1:128, :, :], in_=S3v[0:127, :, :])
                nc.sync.dma_start(out=D3v[0:1, 1:4, :], in_=S3v[127:128, 0:3, :])
                nc.sync.dma_start(
                    out=D3v[0:1, 0:1, :],
                    in_=zrow[:].rearrange("p (q w) -> p q w", w=W2),
                )
                S3 = Sq[:].rearrange("p (q w) -> p q w", w=W2)
                U3 = Upq[:].rearrange("p (q w) -> p q w", w=W2)
                D3 = Dnq[:].rearrange("p (q w) -> p q w", w=W2)
                ax3 = axq[:].rearrange("p (q w) -> p q w", w=N)
                ay3 = ayq[:].rearrange("p (q w) -> p q w", w=N)
                pa3 = pabsq[:].rearrange("p (q w) -> p q w", w=N)

                def qt(tag, dt=F16):
                    t = post.tile([128, NCHUNK * N], dt, tag=tag)
                    return t, t[:].rearrange("p (q w) -> p q w", w=N)

                selq, sel3 = qt("sel")
                pm3q, pm33 = qt("pm3")
                pm2q, pm23 = qt("pm2")
                pm0q, pm03 = qt("pm0")
                mnegq, mneg3 = qt("mneg", U8)
                m0q, m03 = qt("m0", U8)
                m2q, m23 = qt("m2", U8)
                zq, z3 = qt("z")

                TT = nc.vector.tensor_tensor
                TT(out=sel3, in0=D3[:, :, 0:N], in1=U3[:, :, 2 : N + 2],
                   op=mybir.AluOpType.max)
                TT(out=pm33, in0=D3[:, :, 2 : N + 2], in1=U3[:, :, 0:N],
                   op=mybir.AluOpType.max)
                TT(out=pm23, in0=D3[:, :, 1 : N + 1], in1=U3[:, :, 1 : N + 1],
                   op=mybir.AluOpType.max)
                TT(out=pm03, in0=S3[:, :, 0:N], in1=S3[:, :, 2 : N + 2],
                   op=mybir.AluOpType.max)
                nc.vector.scalar_tensor_tensor(
                    out=mneg3, in0=ax3, scalar=float(np.sqrt(2.0)), in1=pa3,
                    op0=mybir.AluOpType.mult, op1=mybir.AluOpType.is_gt,
                )
                nc.vector.scalar_tensor_tensor(
                    out=m03, in0=ax3, scalar=T1, in1=ay3,
                    op0=mybir.AluOpType.mult, op1=mybir.AluOpType.is_ge,
                )
                nc.vector.scalar_tensor_tensor(
                    out=m23, in0=ay3, scalar=T1, in1=ax3,
                    op0=mybir.AluOpType.mult, op1=mybir.AluOpType.is_gt,
                )
                nc.vector.copy_predicated(sel3, mneg3, pm33)
                nc.vector.copy_predicated(sel3, m23, pm23)
                nc.vector.copy_predicated(sel3, m03, pm03)
                oq = outp.tile([128, NCHUNK * N], F16, tag="o")
                o3 = oq[:].rearrange("p (q w) -> p q w", w=N)
                x3 = xh[i][:].rearrange("p (q w) -> p q w", w=N)
                for h0 in (0, 2):
                    hs = slice(h0, h0 + 2)
                    nc.vector.scalar_tensor_tensor(
                        out=z3[:, hs], in0=S3[:, hs, 1 : N + 1], scalar=zs,
                        in1=sel3[:, hs],
                        op0=mybir.AluOpType.mult, op1=mybir.AluOpType.subtract,
                    )
                    nc.vector.scalar_tensor_tensor(
                        out=o3[:, hs], in0=z3[:, hs], scalar=0.0, in1=x3[:, hs],
                        op0=mybir.AluOpType.max, op1=mybir.AluOpType.mult,
                    )
                    for rt in range(h0, h0 + 2):
                        nc.gpsimd.dma_start(
                            out=out_d[i * N + 128 * rt : i * N + 128 * (rt + 1), :],
                            in_=oq[:, N * rt : N * (rt + 1)],
                        )

    nc.compile()
    return nc


# ---------------------------------------------------------------------------
# host side
# ---------------------------------------------------------------------------

def _make_band(weights, offsets, pad):
    M = np.zeros((N, N), dtype=np.float64)
    for w, o in zip(weights, offsets):
        idx = np.arange(N)
        src = idx + o
        if pad == "replicate":
            np.add.at(M, (np.clip(src, 0, N - 1), idx), w)
        else:
            ok = (src >= 0) & (src < N)
            np.add.at(M, (src[ok], idx[ok]), w)
    return M


def _host_weights(gauss_kernel):
    gk = np.asarray(gauss_kernel, dtype=np.float64)[0, 0]
    U, sv, Vt = np.linalg.svd(gk)
    assert sv[1] < 1e-5 * sv[0], "gauss kernel not rank-1 separable"
    wv = U[:, 0] * np.sqrt(sv[0])
    wh = Vt[0] * np.sqrt(sv[0])
    if wv.sum() < 0:
        wv, wh = -wv, -wh
    o5 = [-2, -1, 0, 1, 2]
    o3 = [-1, 0, 1]
    Bv = _make_band(wv, o5, "zero")
    Bh = _make_band(wh, o5, "zero")
    Sv = _make_band([1, 2, 1], o3, "replicate")
    Dv = _make_band([-1, 0, 1], o3, "replicate")
    Sh = _make_band([1, 2, 1], o3, "replicate")
    Dh = _make_band([-1, 0, 1], o3, "replicate")
    f16 = lambda a: np.ascontiguousarray(a, dtype=np.float16)
    return {
        "w_bv": f16(Bv),
        "w_bh": f16(Bh),
        "w_mvx": f16(Bv @ Sv),
        "w_mhx": f16(Bh @ Dh),
        "w_mvy": f16(Bv @ Dv),
        "w_mhy": f16(Bh @ Sh),
    }


_NC_CACHE = {}
LAST_RESULT = None


def kernel(reconst, gauss_kernel, nms_kernel):
    nk = np.asarray(nms_kernel, dtype=np.float64)
    cen = float(nk[0, 0, 1, 1])
    v = float(nk[0, 0, 1, 2])
    # verify nms kernel structure: center + single tap v per direction
    pos = [(1, 2), (2, 2), (2, 1), (2, 0), (1, 0), (0, 0), (0, 1), (0, 2)]
    for d, (r, c) in enumerate(pos):
        k = nk[d, 0].copy()
        assert abs(k[1, 1] - cen) < 1e-6 and abs(k[r, c] - v) < 1e-6
        k[1, 1] = 0.0
        k[r, c] = 0.0
        assert np.abs(k).max() < 1e-7
    assert v < 0

    key = (round(cen, 9), round(v, 9))
    if key not in _NC_CACHE:
        _NC_CACHE[key] = build_nc(cen, v)
    nc = _NC_CACHE[key]

    w = _host_weights(gauss_kernel)
    x = np.asarray(reconst, dtype=np.float32).reshape(B_TOTAL, N, N)
    in_maps = []
    for core in range(N_CORES):
        m = {"x": np.ascontiguousarray(
            x[core * B_CORE : (core + 1) * B_CORE].reshape(B_CORE * N, N)
        )}
        m.update(w)
        in_maps.append(m)

    res = run_bass_kernel_spmd(nc, in_maps, core_ids=list(range(N_CORES)))
    global LAST_RESULT
    LAST_RESULT = res
    out = np.concatenate(
        [r["out"].reshape(B_CORE, 1, N, N) for r in res.results], axis=0
    )
    return out.astype(np.float32)
